# revision 6
# baseline (speedup 1.0000x reference)
"""Trainium2 Bass kernel for nn_MessagePassingConvolution.

Strategy: edges are sorted by receiver and sharded across 8 cores by
contiguous receiver ranges (balanced by edge count), so each core owns a
disjoint slice of output rows and no cross-core reduction is needed.

Per core (all chunks = 128 edges):
  - gather sender node rows from DRAM via indirect DMA
  - edge MLP on the tensor engine (2-way block-diagonal packing for the
    8->64->64->64 layers; final layer uses h2 as the stationary operand so
    `mix` comes out edge-major)
  - CG tensor-product messages + per-irrep gating on the vector engine
  - scatter-add by receiver via one-hot matmul accumulating in PSUM over
    windows of <=128 consecutive receiver nodes
"""

import sys

sys.path.insert(0, "/opt/trn_rl_repo")

import numpy as np

import concourse.bass as bass
import concourse.mybir as mybir
from concourse import bacc
from concourse.tile import TileContext
from concourse.bass_utils import run_bass_kernel_spmd

P = 128
N_NODES = 25000
CHANNELS = 32
HIDDEN = 64
EDGE_DIM = 8
N_CORES = 8
AVG_NEIGH = 16.0

F32 = mybir.dt.float32
I32 = mybir.dt.int32

_PROGRAM_CACHE = {}

# test-harness knobs (grading path leaves these alone)
TRACE = False
TRACE_KW = {}
LAST_EXEC_NS = None
LAST_RESULT = None


# --------------------------------------------------------------------------
# host-side sharding / layout
# --------------------------------------------------------------------------

def _core_split(receivers_sorted):
    """Contiguous node ranges with ~equal edge counts per core."""
    E = receivers_sorted.shape[0]
    bounds = [0]
    for i in range(1, N_CORES):
        target = (E * i) // N_CORES
        node = int(receivers_sorted[min(target, E - 1)])
        bounds.append(min(max(node, bounds[-1] + 1), N_NODES - 1))
    bounds.append(N_NODES)
    return bounds  # node boundaries, len 9


def _make_windows(node_lo, node_hi, deg, t_cap):
    """Greedy windows: <=128 nodes and <= t_cap*128 edges per window.
    Returns list of (node_start, node_end)."""
    cap = t_cap * P
    wins = []
    n = node_lo
    while n < node_hi:
        cnt = 0
        start = n
        while n < node_hi and (n - start) < P:
            d = int(deg[n])
            if cnt + d > cap and cnt > 0:
                break
            cnt += d
            n += 1
        wins.append((start, n))
    return wins


def _prep(node_feats, edge_attrs, edge_feats, senders, receivers):
    E = senders.shape[0]
    order = np.argsort(receivers, kind="stable")
    r_s = receivers[order]
    s_s = senders[order]
    a_s = edge_attrs[order]
    f_s = edge_feats[order]

    deg = np.bincount(receivers, minlength=N_NODES)
    cum = np.concatenate([[0], np.cumsum(deg)])
    bounds = _core_split(r_s)

    # pick T minimizing padded chunks
    best = None
    for t_cap in (14, 15, 16, 17, 18):
        wins_all = [
            _make_windows(bounds[c], bounds[c + 1], deg, t_cap)
            for c in range(N_CORES)
        ]
        nw = max(len(w) for w in wins_all)
        nw += nw % 2
        nc_chunks = nw * t_cap
        if best is None or nc_chunks < best[0]:
            best = (nc_chunks, t_cap, nw, wins_all)
    _, T, NW, wins_all = best
    NC = NW * T
    NCh = NC // 2

    # per-core arrays
    cores = []
    for c in range(N_CORES):
        wins = list(wins_all[c])
        while len(wins) < NW:
            wins.append((bounds[c + 1], bounds[c + 1]))  # empty pad windows

        # chunk order: even windows first, then odd windows
        attrs = np.zeros((NC, P, 8), np.float32)
        sidx = np.zeros((NC, P, 1), np.int32)
        ef = np.zeros((NC, P, EDGE_DIM), np.float32)
        win_starts = np.zeros(NW, np.int64)
        win_lens = np.zeros(NW, np.int64)

        ci = 0  # chunk cursor in kernel order
        for parity in (0, 1):
            for w in range(parity, NW, 2):
                ns, ne = wins[w]
                win_starts[w] = ns
                win_lens[w] = ne - ns
                e0, e1 = int(cum[ns]), int(cum[ne])
                cnt = e1 - e0
                assert cnt <= T * P
                sl = slice(e0, e1)
                flat_a = attrs[ci : ci + T].reshape(T * P, 8)
                flat_i = sidx[ci : ci + T].reshape(T * P, 1)
                flat_f = ef[ci : ci + T].reshape(T * P, EDGE_DIM)
                flat_a[:cnt, 0] = a_s[sl, 0]
                flat_a[:cnt, 1:4] = a_s[sl, 1:4]
                flat_a[:cnt, 4] = (r_s[sl] - ns).astype(np.float32)
                flat_i[:cnt, 0] = s_s[sl]
                flat_f[:cnt] = f_s[sl]
                ci += T
        # kernel order: iterate even-window chunks then odd; but we filled
        # in that order already (parity 0 first).
        ef2 = np.concatenate(
            [
                ef[:NCh].reshape(NCh * P, EDGE_DIM).T,
                ef[NCh:].reshape(NCh * P, EDGE_DIM).T,
            ],
            axis=0,
        )  # [16, NCh*P]
        cores.append(
            dict(
                attrs=np.ascontiguousarray(attrs),
                sidx=np.ascontiguousarray(sidx),
                ef2=np.ascontiguousarray(ef2),
                win_starts=win_starts,
                win_lens=win_lens,
            )
        )

    # node table, j-major layout: col = j*32 + c
    nf = np.ascontiguousarray(
        node_feats.transpose(0, 2, 1).reshape(N_NODES, 4 * CHANNELS)
    ).astype(np.float32)

    return cores, nf, T, NW, NC, NCh


def _prep_weights(W0, W1, W2, W3):
    W0s = (W0 / np.sqrt(np.float32(EDGE_DIM))).astype(np.float32)
    W1s = (W1 / np.sqrt(np.float32(HIDDEN))).astype(np.float32)
    W2s = (W2 / np.sqrt(np.float32(HIDDEN))).astype(np.float32)
    # w3p[h, i*32+c] = W3[h, c*4+i] / sqrt(64) / sqrt(16); i==1 also /sqrt(3)
    W3r = (W3 / np.sqrt(np.float32(HIDDEN)) / np.sqrt(np.float32(AVG_NEIGH)))
    W3r = W3r.reshape(HIDDEN, CHANNELS, 4)
    W3p = np.ascontiguousarray(W3r.transpose(0, 2, 1)).astype(np.float32)
    W3p[:, 1, :] /= np.sqrt(np.float32(3.0))
    W3p = W3p.reshape(HIDDEN, 4 * CHANNELS)
    # duplicate along partitions so rhs base_partition can match lhsT's
    W3p = np.concatenate([W3p, W3p], axis=0)  # [128, 128]

    bd0 = np.zeros((16, 128), np.float32)
    bd0[0:8, 0:64] = W0s
    bd0[8:16, 64:128] = W0s
    bd1 = np.zeros((128, 128), np.float32)
    bd1[0:64, 0:64] = W1s
    bd1[64:128, 64:128] = W1s
    bd2 = np.zeros((128, 128), np.float32)
    bd2[0:64, 0:64] = W2s
    bd2[64:128, 64:128] = W2s
    return bd0, bd1, bd2, W3p


# --------------------------------------------------------------------------
# device program
# --------------------------------------------------------------------------

def _build_program(T, NW, NC, NCh):
    nc = bacc.Bacc()
    Silu = mybir.ActivationFunctionType.Silu
    Copy = mybir.ActivationFunctionType.Copy
    MUL = mybir.AluOpType.mult
    ADD = mybir.AluOpType.add
    SUB = mybir.AluOpType.subtract
    EQ = mybir.AluOpType.is_equal

    nf_d = nc.dram_tensor("nf", [N_NODES, 128], F32, kind="ExternalInput")
    attrs_d = nc.dram_tensor("attrs", [NC, P, 8], F32, kind="ExternalInput")
    sidx_d = nc.dram_tensor("sidx", [NC, P, 1], I32, kind="ExternalInput")
    ef2_d = nc.dram_tensor("ef2", [16, NCh * P], F32, kind="ExternalInput")
    bd0_d = nc.dram_tensor("bd0", [16, 128], F32, kind="ExternalInput")
    bd1_d = nc.dram_tensor("bd1", [128, 128], F32, kind="ExternalInput")
    bd2_d = nc.dram_tensor("bd2", [128, 128], F32, kind="ExternalInput")
    w3p_d = nc.dram_tensor("w3p", [128, 128], F32, kind="ExternalInput")
    out_d = nc.dram_tensor("out", [NW * P, 256], F32, kind="ExternalOutput")

    with TileContext(nc) as tc:
        with (
            tc.tile_pool(name="const", bufs=1) as cpool,
            tc.tile_pool(name="io", bufs=3) as io,
            tc.tile_pool(name="work", bufs=3) as wk,
            tc.tile_pool(name="ps", bufs=2, space="PSUM") as ps,
            tc.tile_pool(name="pagg", bufs=2, space="PSUM") as pagg,
        ):
            bd0_t = cpool.tile([16, 128], F32)
            nc.sync.dma_start(out=bd0_t[:], in_=bd0_d[:, :])
            bd1_t = cpool.tile([128, 128], F32)
            nc.sync.dma_start(out=bd1_t[:], in_=bd1_d[:, :])
            bd2_t = cpool.tile([128, 128], F32)
            nc.sync.dma_start(out=bd2_t[:], in_=bd2_d[:, :])
            w3p_t = cpool.tile([128, 128], F32)
            nc.sync.dma_start(out=w3p_t[:], in_=w3p_d[:, :])
            iota_t = cpool.tile([P, 128], F32)
            nc.gpsimd.iota(
                iota_t[:], [[1, 128]], channel_multiplier=0,
                allow_small_or_imprecise_dtypes=True,
            )

            agg = {}  # parity -> live psum agg tile

            for m in range(NCh):
                # ---------------- MLP for chunk pair (m, m+NCh) -----------
                ef_t = io.tile([16, P], F32, tag="ef")
                nc.sync.dma_start(out=ef_t[:], in_=ef2_d[:, m * P : (m + 1) * P])
                ph = ps.tile([P, P], F32, tag="ph")
                nc.tensor.matmul(out=ph[:], lhsT=bd0_t[:], rhs=ef_t[:],
                                 start=True, stop=True)
                h0 = wk.tile([P, P], F32, tag="h0")
                nc.scalar.activation(out=h0[:], in_=ph[:], func=Silu)
                ph1 = ps.tile([P, P], F32, tag="ph")
                nc.tensor.matmul(out=ph1[:], lhsT=bd1_t[:], rhs=h0[:],
                                 start=True, stop=True)
                h1 = wk.tile([P, P], F32, tag="h1")
                nc.scalar.activation(out=h1[:], in_=ph1[:], func=Silu)
                ph2 = ps.tile([P, P], F32, tag="ph")
                nc.tensor.matmul(out=ph2[:], lhsT=bd2_t[:], rhs=h1[:],
                                 start=True, stop=True)
                h2 = wk.tile([P, P], F32, tag="h2")
                nc.scalar.activation(out=h2[:], in_=ph2[:], func=Silu)

                for half in (0, 1):
                    ch = m + half * NCh  # global chunk index
                    parity = half  # 0 => even windows, 1 => odd
                    wlist_idx = ch // T  # index into kernel window order
                    t_in_w = ch % T
                    w_actual = (
                        2 * wlist_idx if half == 0 else 2 * (wlist_idx - NW // 2) + 1
                    )

                    # mix = h2(half).T @ w3p  -> [128 e, 128 comps] edge-major
                    pmix = ps.tile([P, 128], F32, tag="pmix")
                    nc.tensor.matmul(
                        out=pmix[:],
                        lhsT=h2[64 * half : 64 * half + 64, :],
                        rhs=w3p_t[64 * half : 64 * half + 64, :],
                        start=True, stop=True,
                    )

                    # inputs for this chunk
                    at = io.tile([P, 8], F32, tag="at")
                    nc.sync.dma_start(out=at[:], in_=attrs_d[ch, :, :])
                    si = io.tile([P, 1], I32, tag="si")
                    nc.sync.dma_start(out=si[:], in_=sidx_d[ch, :, :])
                    x = io.tile([P, 128], F32, tag="x")
                    nc.gpsimd.indirect_dma_start(
                        out=x[:], out_offset=None, in_=nf_d[:, :],
                        in_offset=bass.IndirectOffsetOnAxis(ap=si[:, :1], axis=0),
                    )

                    # ---------------- messages ---------------------------
                    msg = wk.tile([P, 256], F32, tag="msg")
                    T1 = wk.tile([P, 128], F32, tag="T1")
                    nc.vector.tensor_scalar(
                        out=T1[:], in0=x[:], scalar1=at[:, 0:1], scalar2=None,
                        op0=MUL,
                    )
                    # k0 = (s*a0)*mix0
                    nc.vector.tensor_tensor(
                        out=msg[:, 0:32], in0=T1[:, 0:32], in1=pmix[:, 0:32],
                        op=MUL,
                    )
                    # k5..7 = (v_j*a0)*mix3
                    for j in (1, 2, 3):
                        nc.vector.tensor_tensor(
                            out=msg[:, 128 + 32 * j : 160 + 32 * j],
                            in0=T1[:, 32 * j : 32 * j + 32],
                            in1=pmix[:, 96:128],
                            op=MUL,
                        )
                    # m0b = sum_j v_j * av_j
                    p3 = wk.tile([P, 96], F32, tag="p3")
                    for j in (1, 2, 3):
                        nc.vector.tensor_scalar(
                            out=p3[:, 32 * (j - 1) : 32 * j],
                            in0=x[:, 32 * j : 32 * j + 32],
                            scalar1=at[:, j : j + 1], scalar2=None, op0=MUL,
                        )
                    m0b = wk.tile([P, 32], F32, tag="m0b")
                    nc.vector.tensor_tensor(
                        out=m0b[:], in0=p3[:, 0:32], in1=p3[:, 32:64], op=ADD
                    )
                    m0b2 = wk.tile([P, 32], F32, tag="m0b2")
                    nc.vector.tensor_tensor(
                        out=m0b2[:], in0=m0b[:], in1=p3[:, 64:96], op=ADD
                    )
                    # k1 = m0b*mix1  (1/sqrt3 folded into w3p)
                    nc.vector.tensor_tensor(
                        out=msg[:, 32:64], in0=m0b2[:], in1=pmix[:, 32:64], op=MUL
                    )
                    # k2..4 = (s*mix2)*av_j
                    s2 = wk.tile([P, 32], F32, tag="s2")
                    nc.vector.tensor_tensor(
                        out=s2[:], in0=x[:, 0:32], in1=pmix[:, 64:96], op=MUL
                    )
                    for j in (1, 2, 3):
                        nc.vector.tensor_scalar(
                            out=msg[:, 32 + 32 * j : 64 + 32 * j],
                            in0=s2[:], scalar1=at[:, j : j + 1], scalar2=None,
                            op0=MUL,
                        )

                    # ---------------- one-hot + scatter -------------------
                    oneh = wk.tile([P, 128], F32, tag="oneh")
                    nc.vector.tensor_scalar(
                        out=oneh[:], in0=iota_t[:], scalar1=at[:, 4:5],
                        scalar2=0.0, op0=SUB, op1=EQ,
                    )
                    if t_in_w == 0:
                        agg[parity] = pagg.tile(
                            [P, 256], F32, tag=f"agg{parity}",
                            name=f"agg{parity}",
                        )
                    nc.tensor.matmul(
                        out=agg[parity][:],
                        lhsT=oneh[:], rhs=msg[:],
                        start=(t_in_w == 0), stop=(t_in_w == T - 1),
                    )
                    if t_in_w == T - 1:
                        ot = wk.tile([P, 256], F32, tag="ot")
                        nc.scalar.activation(
                            out=ot[:], in_=agg[parity][:], func=Copy
                        )
                        nc.sync.dma_start(
                            out=out_d[w_actual * P : (w_actual + 1) * P, :],
                            in_=ot[:],
                        )
    nc.compile()
    return nc


# --------------------------------------------------------------------------
# entry point
# --------------------------------------------------------------------------

def kernel(**inputs):
    node_feats = np.asarray(inputs["node_feats"], np.float32)
    edge_attrs = np.asarray(inputs["edge_attrs"], np.float32)
    edge_feats = np.asarray(inputs["edge_feats"], np.float32)
    senders = np.asarray(inputs["senders"]).astype(np.int64)
    receivers = np.asarray(inputs["receivers"]).astype(np.int64)
    W0 = np.asarray(inputs["W0"], np.float32)
    W1 = np.asarray(inputs["W1"], np.float32)
    W2 = np.asarray(inputs["W2"], np.float32)
    W3 = np.asarray(inputs["W3"], np.float32)

    cores, nf, T, NW, NC, NCh = _prep(
        node_feats, edge_attrs, edge_feats, senders, receivers
    )
    bd0, bd1, bd2, w3p = _prep_weights(W0, W1, W2, W3)

    key = (T, NW, NC, NCh)
    if key not in _PROGRAM_CACHE:
        _PROGRAM_CACHE[key] = _build_program(*key)
    nc = _PROGRAM_CACHE[key]

    in_maps = []
    for c in range(N_CORES):
        in_maps.append(
            {
                "nf": nf,
                "attrs": cores[c]["attrs"],
                "sidx": cores[c]["sidx"],
                "ef2": cores[c]["ef2"],
                "bd0": bd0,
                "bd1": bd1,
                "bd2": bd2,
                "w3p": w3p,
            }
        )

    res = run_bass_kernel_spmd(
        nc, in_maps, core_ids=list(range(N_CORES)), trace=TRACE, **TRACE_KW
    )
    if TRACE:
        global LAST_EXEC_NS, LAST_RESULT
        LAST_EXEC_NS = res.exec_time_ns
        LAST_RESULT = res

    out = np.zeros((N_NODES, CHANNELS, 8), np.float32)
    for c in range(N_CORES):
        r = res.results[c]["out"]  # [NW*P, 256]
        ws = cores[c]["win_starts"]
        wl = cores[c]["win_lens"]
        for w in range(NW):
            L = int(wl[w])
            if L == 0:
                continue
            blk = r[w * P : w * P + L, :].reshape(L, 8, CHANNELS)
            out[int(ws[w]) : int(ws[w]) + L] = blk.transpose(0, 2, 1)
    return out


# revision 9
# speedup vs baseline: 2.0671x; 2.0671x over previous
"""Trainium2 Bass kernel for nn_MessagePassingConvolution.

Strategy: edges are sorted by receiver and sharded across 8 cores by
contiguous receiver ranges (balanced by edge count), so each core owns a
disjoint slice of output rows and no cross-core reduction is needed.

Per core (chunks of 128 edges, processed in groups of 4):
  - gather sender node rows (bf16) from DRAM via indirect DMA, 512 rows
    per instruction
  - edge MLP on the tensor engine in bf16 (2-way block-diagonal packing for
    the 8->64->64->64 layers, batched 512 edges per matmul; final layer uses
    h2 as the stationary operand so `mix` comes out edge-major)
  - CG tensor-product messages + per-irrep gating on the vector engine (bf16)
  - scatter-add by receiver via one-hot matmul (bf16 operands, fp32 PSUM)
    accumulating over windows of <=128 consecutive receiver nodes
"""

import sys

sys.path.insert(0, "/opt/trn_rl_repo")

import numpy as np
import ml_dtypes

import concourse.bass as bass
import concourse.mybir as mybir
from concourse import bacc
from concourse.tile import TileContext
from concourse.bass_utils import run_bass_kernel_spmd

P = 128
N_NODES = 25000
CHANNELS = 32
HIDDEN = 64
EDGE_DIM = 8
N_CORES = 8
AVG_NEIGH = 16.0
GB = 4  # chunks per gather/DMA group and MLP batch

F32 = mybir.dt.float32
BF16 = mybir.dt.bfloat16
I32 = mybir.dt.int32
BF_NP = ml_dtypes.bfloat16

_PROGRAM_CACHE = {}

# test-harness knobs (grading path leaves these alone)
TRACE = False
TRACE_KW = {}
LAST_EXEC_NS = None
LAST_RESULT = None


# --------------------------------------------------------------------------
# host-side sharding / layout
# --------------------------------------------------------------------------

def _core_split(receivers_sorted):
    E = receivers_sorted.shape[0]
    bounds = [0]
    for i in range(1, N_CORES):
        target = (E * i) // N_CORES
        node = int(receivers_sorted[min(target, E - 1)])
        bounds.append(min(max(node, bounds[-1] + 1), N_NODES - 1))
    bounds.append(N_NODES)
    return bounds


def _make_windows(node_lo, node_hi, deg, t_cap):
    cap = t_cap * P
    wins = []
    n = node_lo
    while n < node_hi:
        cnt = 0
        start = n
        while n < node_hi and (n - start) < P:
            d = int(deg[n])
            if cnt + d > cap and cnt > 0:
                break
            cnt += d
            n += 1
        wins.append((start, n))
    return wins


def _prep(node_feats, edge_attrs, edge_feats, senders, receivers):
    order = np.argsort(receivers, kind="stable")
    r_s = receivers[order]
    s_s = senders[order]
    a_s = edge_attrs[order]
    f_s = edge_feats[order]

    deg = np.bincount(receivers, minlength=N_NODES)
    cum = np.concatenate([[0], np.cumsum(deg)])
    bounds = _core_split(r_s)

    best = None
    for t_cap in (14, 15, 16, 17, 18):
        wins_all = [
            _make_windows(bounds[c], bounds[c + 1], deg, t_cap)
            for c in range(N_CORES)
        ]
        nw = max(len(w) for w in wins_all)
        nw += nw % 2
        # need NCh = nw//2 * t_cap divisible by GB
        while ((nw // 2) * t_cap) % GB != 0:
            nw += 2
        nc_chunks = nw * t_cap
        if best is None or nc_chunks < best[0]:
            best = (nc_chunks, t_cap, nw, wins_all)
    _, T, NW, wins_all = best
    NC = NW * T
    NCh = NC // 2

    cores = []
    for c in range(N_CORES):
        wins = list(wins_all[c])
        while len(wins) < NW:
            wins.append((bounds[c + 1], bounds[c + 1]))

        attrs = np.zeros((NC, P, 8), np.float32)
        sidx = np.zeros((NC, P), np.int32)
        ef = np.zeros((NC, P, EDGE_DIM), np.float32)
        win_starts = np.zeros(NW, np.int64)
        win_lens = np.zeros(NW, np.int64)

        ci = 0
        for parity in (0, 1):
            for w in range(parity, NW, 2):
                ns, ne = wins[w]
                win_starts[w] = ns
                win_lens[w] = ne - ns
                e0, e1 = int(cum[ns]), int(cum[ne])
                cnt = e1 - e0
                assert cnt <= T * P
                sl = slice(e0, e1)
                flat_a = attrs[ci : ci + T].reshape(T * P, 8)
                flat_i = sidx[ci : ci + T].reshape(T * P)
                flat_f = ef[ci : ci + T].reshape(T * P, EDGE_DIM)
                flat_a[:cnt, 0] = a_s[sl, 0]
                flat_a[:cnt, 1:4] = a_s[sl, 1:4]
                flat_a[:cnt, 4] = (r_s[sl] - ns).astype(np.float32)
                flat_i[:cnt] = s_s[sl]
                flat_f[:cnt] = f_s[sl]
                ci += T

        # grouped layouts
        NG = NC // GB
        attrs_g = np.ascontiguousarray(
            attrs.reshape(NG, GB, P, 8).transpose(0, 2, 1, 3).reshape(NG, P, GB * 8)
        ).astype(np.float32)
        sidx_g = np.ascontiguousarray(
            sidx.reshape(NG, GB, P).transpose(0, 2, 1)
        )  # [NG, P, GB] int32
        ef2 = np.concatenate(
            [
                ef[:NCh].reshape(NCh * P, EDGE_DIM).T,
                ef[NCh:].reshape(NCh * P, EDGE_DIM).T,
            ],
            axis=0,
        ).astype(BF_NP)  # [16, NCh*P]
        cores.append(
            dict(
                attrs=attrs_g,
                sidx=sidx_g,
                ef2=np.ascontiguousarray(ef2),
                win_starts=win_starts,
                win_lens=win_lens,
            )
        )

    # node table, j-major layout: col = j*32 + c, bf16
    nf = np.ascontiguousarray(
        node_feats.transpose(0, 2, 1).reshape(N_NODES, 4 * CHANNELS)
    ).astype(BF_NP)

    return cores, nf, T, NW, NC, NCh


def _prep_weights(W0, W1, W2, W3):
    W0s = W0 / np.sqrt(np.float32(EDGE_DIM))
    W1s = W1 / np.sqrt(np.float32(HIDDEN))
    W2s = W2 / np.sqrt(np.float32(HIDDEN))
    W3r = W3 / np.sqrt(np.float32(HIDDEN)) / np.sqrt(np.float32(AVG_NEIGH))
    W3r = W3r.reshape(HIDDEN, CHANNELS, 4)
    W3p = np.ascontiguousarray(W3r.transpose(0, 2, 1)).astype(np.float32)
    W3p[:, 1, :] /= np.sqrt(np.float32(3.0))
    W3p = W3p.reshape(HIDDEN, 4 * CHANNELS)
    W3p = np.concatenate([W3p, W3p], axis=0)  # [128, 128], dup for base_partition

    bd0 = np.zeros((16, 128), np.float32)
    bd0[0:8, 0:64] = W0s
    bd0[8:16, 64:128] = W0s
    bd1 = np.zeros((128, 128), np.float32)
    bd1[0:64, 0:64] = W1s
    bd1[64:128, 64:128] = W1s
    bd2 = np.zeros((128, 128), np.float32)
    bd2[0:64, 0:64] = W2s
    bd2[64:128, 64:128] = W2s
    return (
        bd0.astype(BF_NP),
        bd1.astype(BF_NP),
        bd2.astype(BF_NP),
        W3p.astype(BF_NP),
    )


# --------------------------------------------------------------------------
# device program
# --------------------------------------------------------------------------

def _build_program(T, NW, NC, NCh):
    nc = bacc.Bacc()
    Silu = mybir.ActivationFunctionType.Silu
    Copy = mybir.ActivationFunctionType.Copy
    MUL = mybir.AluOpType.mult
    ADD = mybir.AluOpType.add
    SUB = mybir.AluOpType.subtract
    EQ = mybir.AluOpType.is_equal
    NG = NC // GB

    nf_d = nc.dram_tensor("nf", [N_NODES, 128], BF16, kind="ExternalInput")
    attrs_d = nc.dram_tensor("attrs", [NG, P, GB * 8], F32, kind="ExternalInput")
    sidx_d = nc.dram_tensor("sidx", [NG, P, GB], I32, kind="ExternalInput")
    ef2_d = nc.dram_tensor("ef2", [16, NCh * P], BF16, kind="ExternalInput")
    bd0_d = nc.dram_tensor("bd0", [16, 128], BF16, kind="ExternalInput")
    bd1_d = nc.dram_tensor("bd1", [128, 128], BF16, kind="ExternalInput")
    bd2_d = nc.dram_tensor("bd2", [128, 128], BF16, kind="ExternalInput")
    w3p_d = nc.dram_tensor("w3p", [128, 128], BF16, kind="ExternalInput")
    out_d = nc.dram_tensor("out", [NW * P, 256], F32, kind="ExternalOutput")

    with TileContext(nc) as tc:
        with (
            tc.tile_pool(name="const", bufs=1) as cpool,
            tc.tile_pool(name="io", bufs=3) as io,
            tc.tile_pool(name="wk", bufs=3) as wk,
            tc.tile_pool(name="ps", bufs=2, space="PSUM") as ps,
            tc.tile_pool(name="pagg", bufs=2, space="PSUM") as pagg,
        ):
            bd0_t = cpool.tile([16, 128], BF16)
            nc.sync.dma_start(out=bd0_t[:], in_=bd0_d[:, :])
            bd1_t = cpool.tile([128, 128], BF16)
            nc.sync.dma_start(out=bd1_t[:], in_=bd1_d[:, :])
            bd2_t = cpool.tile([128, 128], BF16)
            nc.sync.dma_start(out=bd2_t[:], in_=bd2_d[:, :])
            w3p_t = cpool.tile([128, 128], BF16)
            nc.sync.dma_start(out=w3p_t[:], in_=w3p_d[:, :])
            iota_t = cpool.tile([P, 128], BF16)
            nc.gpsimd.iota(
                iota_t[:], [[1, 128]], channel_multiplier=0,
                allow_small_or_imprecise_dtypes=True,
            )

            agg = {}

            for b in range(NCh // GB):
                ga = b
                gb_ = NCh // GB + b
                xg = {}
                atg = {}
                for half, g in ((0, ga), (1, gb_)):
                    si4 = io.tile([P, GB], I32, tag=f"si{half}", name=f"si{half}")
                    nc.sync.dma_start(out=si4[:], in_=sidx_d[g, :, :])
                    at4 = io.tile(
                        [P, GB * 8], F32, tag=f"at{half}", name=f"at{half}"
                    )
                    nc.sync.dma_start(out=at4[:], in_=attrs_d[g, :, :])
                    at4b = io.tile(
                        [P, GB * 8], BF16, tag=f"at{half}b", name=f"at{half}b"
                    )
                    nc.vector.tensor_copy(out=at4b[:], in_=at4[:])
                    x4 = io.tile([P, GB, 128], BF16, tag=f"x{half}", name=f"x{half}")
                    for kk in range(GB):
                        nc.gpsimd.indirect_dma_start(
                            out=x4[:, kk, :], out_offset=None, in_=nf_d[:, :],
                            in_offset=bass.IndirectOffsetOnAxis(
                                ap=si4[:, kk : kk + 1], axis=0
                            ),
                        )
                    xg[half] = x4
                    atg[half] = at4
                    atg[(half, 'b')] = at4b

                # -------- MLP batch: GB iters = 2*GB chunks ----------------
                ef_t = io.tile([16, GB * P], BF16, tag="ef")
                nc.sync.dma_start(
                    out=ef_t[:], in_=ef2_d[:, b * GB * P : (b + 1) * GB * P]
                )
                ph0 = ps.tile([P, GB * P], F32, tag="ph")
                nc.tensor.matmul(out=ph0[:], lhsT=bd0_t[:], rhs=ef_t[:],
                                 start=True, stop=True)
                h0 = wk.tile([P, GB * P], BF16, tag="h0")
                nc.scalar.activation(out=h0[:], in_=ph0[:], func=Silu)
                ph1 = ps.tile([P, GB * P], F32, tag="ph")
                nc.tensor.matmul(out=ph1[:], lhsT=bd1_t[:], rhs=h0[:],
                                 start=True, stop=True)
                h1 = wk.tile([P, GB * P], BF16, tag="h1")
                nc.scalar.activation(out=h1[:], in_=ph1[:], func=Silu)
                ph2 = ps.tile([P, GB * P], F32, tag="ph")
                nc.tensor.matmul(out=ph2[:], lhsT=bd2_t[:], rhs=h1[:],
                                 start=True, stop=True)
                h2 = wk.tile([P, GB * P], BF16, tag="h2")
                nc.scalar.activation(out=h2[:], in_=ph2[:], func=Silu)

                for k in range(GB):
                    m = GB * b + k
                    for half in (0, 1):
                        ch = m + half * NCh
                        wlist_idx = ch // T
                        t_in_w = ch % T
                        w_actual = (
                            2 * wlist_idx
                            if half == 0
                            else 2 * (wlist_idx - NW // 2) + 1
                        )

                        pmix = ps.tile([P, 128], F32, tag="pmix")
                        nc.tensor.matmul(
                            out=pmix[:],
                            lhsT=h2[64 * half : 64 * half + 64,
                                    k * P : (k + 1) * P],
                            rhs=w3p_t[64 * half : 64 * half + 64, :],
                            start=True, stop=True,
                        )
                        em = wk.tile([P, 128], BF16, tag="em")
                        nc.scalar.activation(out=em[:], in_=pmix[:], func=Copy)

                        x = xg[half][:, k, :]          # [128,128] bf16
                        at = atg[half][:, k * 8 : (k + 1) * 8]   # [128,8] f32
                        atb = atg[(half, 'b')][:, k * 8 : (k + 1) * 8]  # bf16

                        msg = wk.tile([P, 256], BF16, tag="msg")
                        T1 = wk.tile([P, 128], BF16, tag="T1")
                        nc.scalar.activation(
                            out=T1[:], in_=x, func=Copy, scale=at[:, 0:1]
                        )
                        # k0 = (s*a0)*mix0
                        nc.vector.tensor_tensor(
                            out=msg[:, 0:32], in0=T1[:, 0:32],
                            in1=em[:, 0:32], op=MUL,
                        )
                        # k5..7 = (v_j*a0)*mix3 -- one op, mix3 bcast over j
                        nc.vector.tensor_tensor(
                            out=msg[:, 160:256].rearrange(
                                "p (j c) -> p j c", j=3
                            ),
                            in0=T1[:, 32:128].rearrange("p (j c) -> p j c", j=3),
                            in1=em[:, 96:128].unsqueeze(1).broadcast_to(
                                [P, 3, 32]
                            ),
                            op=MUL,
                        )
                        # p3_j = v_j * av_j  -- one op, av bcast over c
                        p3 = wk.tile([P, 96], BF16, tag="p3")
                        nc.vector.tensor_tensor(
                            out=p3[:].rearrange("p (j c) -> p j c", j=3),
                            in0=x[:, 32:128].rearrange("p (j c) -> p j c", j=3),
                            in1=atb[:, 1:4].unsqueeze(2).broadcast_to(
                                [P, 3, 32]
                            ),
                            op=MUL,
                        )
                        m0b = wk.tile([P, 32], BF16, tag="m0b")
                        nc.vector.tensor_tensor(
                            out=m0b[:], in0=p3[:, 0:32], in1=p3[:, 32:64], op=ADD
                        )
                        m0b2 = wk.tile([P, 32], BF16, tag="m0b2")
                        nc.vector.tensor_tensor(
                            out=m0b2[:], in0=m0b[:], in1=p3[:, 64:96], op=ADD
                        )
                        nc.vector.tensor_tensor(
                            out=msg[:, 32:64], in0=m0b2[:], in1=em[:, 32:64],
                            op=MUL,
                        )
                        # k2..4 = (s*mix2)*av_j
                        s2 = wk.tile([P, 32], BF16, tag="s2")
                        nc.vector.tensor_tensor(
                            out=s2[:], in0=x[:, 0:32], in1=em[:, 64:96], op=MUL
                        )
                        nc.vector.tensor_tensor(
                            out=msg[:, 64:160].rearrange("p (j c) -> p j c", j=3),
                            in0=s2[:].unsqueeze(1).broadcast_to([P, 3, 32]),
                            in1=atb[:, 1:4].unsqueeze(2).broadcast_to(
                                [P, 3, 32]
                            ),
                            op=MUL,
                        )

                        oneh = wk.tile([P, 128], BF16, tag="oneh")
                        nc.vector.tensor_scalar(
                            out=oneh[:], in0=iota_t[:], scalar1=at[:, 4:5],
                            scalar2=0.0, op0=SUB, op1=EQ,
                        )
                        if t_in_w == 0:
                            agg[half] = pagg.tile(
                                [P, 256], F32, tag=f"agg{half}",
                                name=f"agg{half}",
                            )
                        nc.tensor.matmul(
                            out=agg[half][:], lhsT=oneh[:], rhs=msg[:],
                            start=(t_in_w == 0), stop=(t_in_w == T - 1),
                        )
                        if t_in_w == T - 1:
                            ot = wk.tile([P, 256], F32, tag="ot")
                            nc.scalar.activation(
                                out=ot[:], in_=agg[half][:], func=Copy
                            )
                            nc.sync.dma_start(
                                out=out_d[w_actual * P : (w_actual + 1) * P, :],
                                in_=ot[:],
                            )
    nc.compile()
    return nc


# --------------------------------------------------------------------------
# entry point
# --------------------------------------------------------------------------

def kernel(**inputs):
    node_feats = np.asarray(inputs["node_feats"], np.float32)
    edge_attrs = np.asarray(inputs["edge_attrs"], np.float32)
    edge_feats = np.asarray(inputs["edge_feats"], np.float32)
    senders = np.asarray(inputs["senders"]).astype(np.int64)
    receivers = np.asarray(inputs["receivers"]).astype(np.int64)
    W0 = np.asarray(inputs["W0"], np.float32)
    W1 = np.asarray(inputs["W1"], np.float32)
    W2 = np.asarray(inputs["W2"], np.float32)
    W3 = np.asarray(inputs["W3"], np.float32)

    cores, nf, T, NW, NC, NCh = _prep(
        node_feats, edge_attrs, edge_feats, senders, receivers
    )
    bd0, bd1, bd2, w3p = _prep_weights(W0, W1, W2, W3)

    key = (T, NW, NC, NCh)
    if key not in _PROGRAM_CACHE:
        _PROGRAM_CACHE[key] = _build_program(*key)
    nc = _PROGRAM_CACHE[key]

    in_maps = []
    for c in range(N_CORES):
        in_maps.append(
            {
                "nf": nf,
                "attrs": cores[c]["attrs"],
                "sidx": cores[c]["sidx"],
                "ef2": cores[c]["ef2"],
                "bd0": bd0,
                "bd1": bd1,
                "bd2": bd2,
                "w3p": w3p,
            }
        )

    res = run_bass_kernel_spmd(
        nc, in_maps, core_ids=list(range(N_CORES)), trace=TRACE, **TRACE_KW
    )
    if TRACE:
        global LAST_EXEC_NS, LAST_RESULT
        LAST_EXEC_NS = res.exec_time_ns
        LAST_RESULT = res

    out = np.zeros((N_NODES, CHANNELS, 8), np.float32)
    for c in range(N_CORES):
        r = res.results[c]["out"]
        ws = cores[c]["win_starts"]
        wl = cores[c]["win_lens"]
        for w in range(NW):
            L = int(wl[w])
            if L == 0:
                continue
            blk = r[w * P : w * P + L, :].reshape(L, 8, CHANNELS)
            out[int(ws[w]) : int(ws[w]) + L] = blk.transpose(0, 2, 1)
    return out
